# revision 1
# baseline (speedup 1.0000x reference)
"""Trainium2 Bass kernel: 2-layer LSTM decoder with embedding lookup.

Reference computation (per nn.Decoder):
    tgt_embed = emb[prev_tgt_tokens]                      # [B, T, D]
    for t in 0..T-1:
        x = tgt_embed[:, t]
        for l in 0..1:
            gates = x @ W_ih[l].T + b_ih[l] + h[l] @ W_hh[l].T + b_hh[l]
            i, f, g, o = split(gates, 4)
            c[l] = sigmoid(f) * c[l] + sigmoid(i) * tanh(g)
            h[l] = sigmoid(o) * tanh(c[l])
            x = h[l]
        out[:, t] = h[1]

Sharding: data-parallel over batch B=64 across 8 cores (8 rows each);
embedding + LSTM weights replicated; the sequential time loop runs
on-device per core, fully unrolled.

Kernel design (per core):
  - Embedding gather via indirect DMA (128 rows per call), PE-transposed
    into K-major layout.
  - Input projection x @ W_ih[0].T batched over all T steps as one big
    matmul, spilled to a DRAM scratch and staged back per step.
  - Recurrent loop: gates in [8(batch part), 2048(free)] layout; the
    h @ W_hh.T matmuls keep h^T as the (tiny) stationary operand and
    stream W^T as moving data, which is the fp32 throughput-optimal
    orientation. Layer-0 of step t is interleaved with layer-1 of step
    t-1 so the activation chains hide under PE work.
"""

import os

import numpy as np

import concourse.bass as bass
import concourse.mybir as mybir
import concourse.tile as tile
from concourse import bacc
from concourse.bass_utils import run_bass_kernel_spmd
from concourse.masks import make_identity

N_CORES = 8
B = 64
T = int(os.environ.get("BASS_LSTM_T", "128"))
D = 512
V = 32000
G = 4 * D            # 2048 gate dims per layer
BL = B // N_CORES    # 8 batch rows per core
KC = D // 128        # 4 contraction chunks of 128
NB = G // 512        # 4 PSUM banks of 512 per gate vector
MT = BL * T // 128   # M-tiles (128 token rows each) for the input matmul
REPS = int(os.environ.get("BASS_LSTM_REPS", "1"))  # timing-only: loop phase B
ABLATE = os.environ.get("BASS_ABLATE", "")  # "", "mmonly", "notrans" (sim experiments)
F32 = mybir.dt.float32
I32 = mybir.dt.int32
AFT = mybir.ActivationFunctionType

# Matmul compute dtype. float32r is the fast fp32 PE mode: 1 cycle/row for
# moving free dim >= 256 vs 4 cycles/row for plain fp32 (2 half-speed passes).
# The BIR verifier requires fp32r matmul operands to be produced by an
# instruction that rounds to fp32r, so operand tiles are declared fp32r and
# filled via converting DVE copies.
WDT = {
    "f32": F32,
    "f32r": mybir.dt.float32r,
}[os.environ.get("BASS_LSTM_MMDT", "f32r")]

# gate banks after host-side permutation: [f, i, g, o]
BANK_F, BANK_I, BANK_G, BANK_O = 0, 1, 2, 3
R1 = 32  # partition row where the layer-1 lane starts (32-aligned for PE)


def _nsl(n):
    return slice(n * 512, (n + 1) * 512)


def _build():
    nc = bacc.Bacc(
        "TRN2",
        target_bir_lowering=False,
        debug=False,
        enable_asserts=False,
        num_devices=N_CORES,
    )

    tok_d = nc.dram_tensor("tokens", [BL * T, 1], I32, kind="ExternalInput")
    emb_d = nc.dram_tensor("emb", [V, D], F32, kind="ExternalInput")
    wih0_d = nc.dram_tensor("wih0t", [D, G], F32, kind="ExternalInput")
    whh0_d = nc.dram_tensor("whh0t", [D, G], F32, kind="ExternalInput")
    wih1_d = nc.dram_tensor("wih1t", [D, G], F32, kind="ExternalInput")
    whh1_d = nc.dram_tensor("whh1t", [D, G], F32, kind="ExternalInput")
    bias_d = nc.dram_tensor("bias", [2, 128, G], F32, kind="ExternalInput")
    ht_d = nc.dram_tensor("ht_init", [2, 128, KC * BL], F32, kind="ExternalInput")
    c_d = nc.dram_tensor("c_init", [2, BL, D], F32, kind="ExternalInput")
    out_d = nc.dram_tensor("out", [BL, T, D], F32, kind="ExternalOutput")

    with tile.TileContext(nc) as tc:
        _body(
            tc,
            tok=tok_d.ap(),
            emb=emb_d.ap(),
            w=[wih0_d.ap(), whh0_d.ap(), wih1_d.ap(), whh1_d.ap()],
            bias=bias_d.ap(),
            ht0=ht_d.ap(),
            c0=c_d.ap(),
            out=out_d.ap(),
        )
    nc.compile()
    return nc


def _body(tc, tok, emb, w, bias, ht0, c0, out):
    nc = tc.nc
    with (
        tc.tile_pool(name="wpool", bufs=1) as wp,
        tc.tile_pool(name="dram", bufs=1, space="DRAM") as dr,
        tc.tile_pool(name="state", bufs=1) as st,
        tc.tile_pool(name="work", bufs=2) as wk,
        tc.tile_pool(name="pspool", bufs=4, space="PSUM") as pp,
    ):
        # ---- persistent tiles -------------------------------------------
        id_sb = wp.tile([128, 128], F32)
        make_identity(nc, id_sb[:])

        whh0_sb = wp.tile([128, KC * G], WDT)
        wih1_sb = wp.tile([128, KC * G], WDT)
        whh1_sb = wp.tile([128, KC * G], WDT)

        def load_w(dst, src_ap, ci):
            # DMA one K-chunk to an fp32 stage, then convert-copy into the
            # fp32r-typed resident tile.
            for c in range(KC):
                stg = wk.tile(
                    [128, G], F32, tag=f"g{(ci + c) % 2}", bufs=1, name="wstage"
                )
                nc.sync.dma_start(
                    out=stg[:],
                    in_=src_ap.rearrange("(c p) n -> p c n", p=128)[:, c, :],
                )
                nc.vector.tensor_copy(out=dst[:, c * G : (c + 1) * G], in_=stg[:])

        load_w(whh0_sb, w[1], 0)
        load_w(wih1_sb, w[2], 1)
        load_w(whh1_sb, w[3], 0)

        bias1_sb = wp.tile([BL, G], F32)
        nc.sync.dma_start(out=bias1_sb[:], in_=bias[1, :BL, :])

        bias1_sb = wp.tile([BL, G], F32)
        nc.sync.dma_start(out=bias1_sb[:], in_=bias[1, :BL, :])

        # input projection for all steps, spilled to DRAM scratch
        gx_dram = dr.tile([MT * 128, G], F32)

        # LSTM state + persistent chain tiles. Layer 0 (step t) lives on
        # partitions 0:8, layer 1 (step t-1) on partitions 32:40 ("lanes"),
        # so one elementwise op handles both layers (DVE/ACT cost scales
        # with the free dim only). Rows between the lanes hold junk that is
        # memset once and never published.
        NR = R1 + BL  # 40 partition rows
        hT = [None, None]  # [128, KC*BL], h^T packed
        for l in range(2):
            hstg = wk.tile([128, KC * BL], F32, tag="h", name="hstg")
            nc.sync.dma_start(out=hstg[:], in_=ht0[l])
            t0 = st.tile([128, KC * BL], WDT, tag=f"ht{l}", bufs=2)
            nc.vector.tensor_copy(out=t0[:], in_=hstg[:])
            hT[l] = t0

        cst = st.tile([NR, D], F32)
        gt = st.tile([NR, G], F32)
        fct = st.tile([NR, D], F32)
        mt_ = st.tile([NR, D], F32)
        tch = st.tile([NR, D], F32)
        hst = st.tile([NR, D], F32)
        for tile_ in (cst, gt, fct, mt_, tch, hst):
            nc.vector.memset(tile_[:], 0.0)
        nc.sync.dma_start(out=cst[:BL, :], in_=c0[0])
        nc.sync.dma_start(out=cst[R1 : R1 + BL, :], in_=c0[1])

        # ---- phase A: gather + transpose + batched input projection ----
        with tc.tile_pool(name="ph0", bufs=1) as p0:
            wih0_sb = p0.tile([128, KC * G], WDT)
            load_w(wih0_sb, w[0], 1)
            bias0_bc = p0.tile([128, G], F32)
            nc.sync.dma_start(out=bias0_bc[:], in_=bias[0])

            for m in range(MT):
                idx_m = p0.tile([128, 1], I32, tag="idx", bufs=2)
                nc.sync.dma_start(out=idx_m[:], in_=tok[m * 128 : (m + 1) * 128, :])
                emb_m = p0.tile([128, D], F32, tag="embrows", bufs=1)
                nc.gpsimd.indirect_dma_start(
                    out=emb_m[:],
                    out_offset=None,
                    in_=emb,
                    in_offset=bass.IndirectOffsetOnAxis(ap=idx_m[:, :1], axis=0),
                )
                # transpose [tb, d] -> [d, tb] per 128-chunk of d
                pst = pp.tile([128, D], F32, tag="ps")
                for c in range(KC):
                    nc.tensor.transpose(
                        out=pst[:, c * 128 : (c + 1) * 128],
                        in_=emb_m[:, c * 128 : (c + 1) * 128],
                        identity=id_sb[:],
                    )
                embT_m = p0.tile([128, D], WDT, tag="embT", bufs=1)
                for c in range(KC):
                    nc.vector.tensor_copy(
                        out=embT_m[:, c * 128 : (c + 1) * 128],
                        in_=pst[:, c * 128 : (c + 1) * 128],
                    )
                # batched input matmul for this M-tile (per-bank psum slots)
                gxs = wk.tile([128, G], F32, tag="g0", bufs=1, name="gxs")
                for n in range(NB):
                    psm = pp.tile([128, 512], F32, tag="ps", name="psm")
                    for c in range(KC):
                        nc.tensor.matmul(
                            out=psm[:, :],
                            lhsT=embT_m[:, c * 128 : (c + 1) * 128],
                            rhs=wih0_sb[:, c * G + n * 512 : c * G + (n + 1) * 512],
                            start=(c == 0),
                            stop=(c == KC - 1),
                        )
                    nc.vector.tensor_add(
                        out=gxs[:, _nsl(n)], in0=psm[:, :], in1=bias0_bc[:, _nsl(n)]
                    )
                nc.sync.dma_start(
                    out=gx_dram[m * 128 : (m + 1) * 128, :], in_=gxs[:]
                )

        # ---- phase B: recurrent loop ------------------------------------
        # Iteration t emits layer-0 matmuls for step t and layer-1 matmuls
        # for step t-1 into shared per-bank PSUM tiles (lanes 0:8 / 32:40),
        # then one stacked activation chain for both. All activations are
        # Sigmoid (tanh(x) = 2*sigmoid(2x) - 1) so the ACT engine never
        # reloads its function table (1.3us per switch).
        FI, GSL, OSL = slice(0, 1024), _nsl(BANK_G), _nsl(BANK_O)

        def zero_psum():
            # One-time scrub so stacked chain ops can read the junk rows
            # between the lanes without tripping finite-checks.
            for _ in range(4):
                z = pp.tile([128, 1024], F32, tag="ps", name="pz")
                nc.vector.memset(z[:], 0.0)

        zero_psum()

        def mm_group(pb, col0, stat, w_sb, n, start, stop, rows=slice(0, BL)):
            for c in range(KC):
                nc.tensor.matmul(
                    out=pb[rows, col0 : col0 + 512],
                    lhsT=stat[:, c * BL : (c + 1) * BL],
                    rhs=w_sb[:, c * G + n * 512 : c * G + (n + 1) * 512],
                    start=start and c == 0,
                    stop=stop and c == KC - 1,
                )

        for rep in range(REPS):
          for t in range(T + 1):
            last = t == T
            first = t == 0
            gxt = None
            if not last:
                gxt = wk.tile([BL, G], F32, tag="gxt", bufs=3)
                nc.sync.dma_start(
                    out=gxt[:], in_=gx_dram[t * BL : (t + 1) * BL, :]
                )

            # matmuls: separate per-lane psum tiles so each bank+lane is one
            # uninterrupted accumulation group (keeps the PE ramp warm)
            pb_fi0 = pb_go0 = pb_fi1 = pb_go1 = None
            if not last:
                pb_fi0 = pp.tile([128, 1024], F32, tag="ps", name="pb_fi0")
                pb_go0 = pp.tile([128, 1024], F32, tag="ps", name="pb_go0")
            if not first:
                pb_fi1 = pp.tile([128, 1024], F32, tag="ps", name="pb_fi1")
                pb_go1 = pp.tile([128, 1024], F32, tag="ps", name="pb_go1")
            for bi, (p0_, p1_, col0) in enumerate(
                [
                    (pb_fi0, pb_fi1, 0),
                    (pb_fi0, pb_fi1, 512),
                    (pb_go0, pb_go1, 0),
                    (pb_go0, pb_go1, 512),
                ]
            ):
                if not last:
                    mm_group(p0_, col0, hT[0], whh0_sb, bi, True, True, slice(0, BL))
                if not first:
                    mm_group(p1_, col0, hT[0], wih1_sb, bi, True, False, slice(0, BL))
                    mm_group(p1_, col0, hT[1], whh1_sb, bi, False, True, slice(0, BL))

            # per-lane gate adds (lane inputs differ), stacked everything else
            L0, L1 = slice(0, BL), slice(R1, R1 + BL)
            if not last:
                nc.vector.tensor_add(
                    out=gt[L0, FI], in0=pb_fi0[:BL, :], in1=gxt[:, FI]
                )
            if not first:
                nc.vector.tensor_add(
                    out=gt[L1, FI], in0=pb_fi1[:BL, :], in1=bias1_sb[:, FI]
                )
            nc.scalar.activation(out=gt[:, FI], in_=gt[:, FI], func=AFT.Sigmoid)
            nc.vector.tensor_mul(out=fct[:], in0=gt[:, _nsl(BANK_F)], in1=cst[:])
            # fi_diff = f*c - sig_i  (the "- i" term of i*(2s_g - 1))
            nc.vector.tensor_sub(out=fct[:], in0=fct[:], in1=gt[:, _nsl(BANK_I)])
            if not last:
                nc.vector.tensor_add(
                    out=gt[L0, GSL], in0=pb_go0[:BL, 0:512], in1=gxt[:, GSL]
                )
            if not first:
                nc.vector.tensor_add(
                    out=gt[L1, GSL], in0=pb_go1[:BL, 0:512], in1=bias1_sb[:, GSL]
                )
            nc.scalar.activation(
                out=gt[:, GSL], in_=gt[:, GSL], func=AFT.Sigmoid, scale=2.0
            )
            # m = 2 * sig_g * sig_i ; c = fi_diff + m
            nc.vector.scalar_tensor_tensor(
                out=mt_[:], in0=gt[:, GSL], scalar=2.0, in1=gt[:, _nsl(BANK_I)],
                op0=mybir.AluOpType.mult, op1=mybir.AluOpType.mult,
            )
            crows = slice(0, BL) if first else (slice(R1, R1 + BL) if last
                                                else slice(0, NR))
            nc.vector.tensor_add(out=cst[crows, :], in0=fct[crows, :],
                                 in1=mt_[crows, :])
            # tanh(c) = 2*sigmoid(2c) - 1
            nc.scalar.activation(out=tch[:], in_=cst[:], func=AFT.Sigmoid, scale=2.0)
            nc.vector.tensor_scalar(
                out=tch[:], in0=tch[:], scalar1=2.0, scalar2=-1.0,
                op0=mybir.AluOpType.mult, op1=mybir.AluOpType.add,
            )
            if not last:
                nc.vector.tensor_add(
                    out=gt[L0, OSL], in0=pb_go0[:BL, 512:1024], in1=gxt[:, OSL]
                )
            if not first:
                nc.vector.tensor_add(
                    out=gt[L1, OSL], in0=pb_go1[:BL, 512:1024], in1=bias1_sb[:, OSL]
                )
            nc.scalar.activation(out=gt[:, OSL], in_=gt[:, OSL], func=AFT.Sigmoid)
            nc.vector.tensor_mul(out=hst[:], in0=gt[:, OSL], in1=tch[:])
            if not first:
                nc.sync.dma_start(out=out[:, t - 1, :], in_=hst[R1 : R1 + BL, :])

            # h -> h^T transposes into the consumed G-regions + f32r copies
            if not last:
                for c in range(KC):
                    nc.tensor.transpose(
                        out=pb_go0[:, c * BL : (c + 1) * BL],
                        in_=hst[:BL, c * 128 : (c + 1) * 128],
                        identity=id_sb[:BL, :BL],
                    )
                hT0n = st.tile([128, KC * BL], WDT, tag="ht0", bufs=2, name="hT0n")
                nc.vector.tensor_copy(out=hT0n[:], in_=pb_go0[:, 0 : KC * BL])
                hT[0] = hT0n
                if not first:
                    for c in range(KC):
                        nc.tensor.transpose(
                            out=pb_go1[:, c * BL : (c + 1) * BL],
                            in_=hst[R1 : R1 + BL, c * 128 : (c + 1) * 128],
                            identity=id_sb[R1 : R1 + BL, R1 : R1 + BL],
                        )
                    hT1n = st.tile(
                        [128, KC * BL], WDT, tag="ht1", bufs=2, name="hT1n"
                    )
                    nc.vector.tensor_copy(out=hT1n[:], in_=pb_go1[:, 0 : KC * BL])
                    hT[1] = hT1n


_NC_CACHE = {}


def _get_nc():
    if "nc" not in _NC_CACHE:
        _NC_CACHE["nc"] = _build()
    return _NC_CACHE["nc"]


def _make_in_maps(inputs):
    tokens = np.asarray(inputs["prev_tgt_tokens"])[:, :T].astype(np.int32)  # [B, T]
    emb = np.ascontiguousarray(np.asarray(inputs["emb"], dtype=np.float32))
    W_ih = np.asarray(inputs["W_ih"], dtype=np.float32)
    W_hh = np.asarray(inputs["W_hh"], dtype=np.float32)
    b_ih = np.asarray(inputs["b_ih"], dtype=np.float32)
    b_hh = np.asarray(inputs["b_hh"], dtype=np.float32)
    hiddens = np.asarray(inputs["hiddens"], dtype=np.float32)
    cells = np.asarray(inputs["cells"], dtype=np.float32)

    def permute_gates(a, axis):
        # PyTorch gate order [i, f, g, o] -> kernel bank order [f, i, g, o]
        blocks = np.split(a, 4, axis=axis)
        return np.concatenate([blocks[1], blocks[0], blocks[2], blocks[3]], axis=axis)

    wih0t = np.ascontiguousarray(permute_gates(W_ih[0].T, 1))  # [D, G]
    whh0t = np.ascontiguousarray(permute_gates(W_hh[0].T, 1))
    wih1t = np.ascontiguousarray(permute_gates(W_ih[1].T, 1))
    whh1t = np.ascontiguousarray(permute_gates(W_hh[1].T, 1))
    bias = np.ascontiguousarray(
        np.broadcast_to(
            permute_gates(b_ih + b_hh, 1)[:, None, :], (2, 128, G)
        ).astype(np.float32)
    )

    in_maps = []
    for core in range(N_CORES):
        sl = slice(core * BL, (core + 1) * BL)
        tok_tm = np.ascontiguousarray(tokens[sl].T.reshape(BL * T, 1))  # t-major
        ht = np.empty((2, 128, KC * BL), dtype=np.float32)
        for l in range(2):
            # [BL, D] -> h^T [D, BL] -> [KC, 128, BL] -> [128, KC, BL]
            htl = hiddens[l, sl].T.reshape(KC, 128, BL).transpose(1, 0, 2)
            ht[l] = htl.reshape(128, KC * BL)
        cin = np.ascontiguousarray(cells[:, sl, :])
        in_maps.append(
            {
                "tokens": tok_tm,
                "emb": emb,
                "wih0t": wih0t,
                "whh0t": whh0t,
                "wih1t": wih1t,
                "whh1t": whh1t,
                "bias": bias,
                "ht_init": np.ascontiguousarray(ht),
                "c_init": cin,
            }
        )
    return in_maps


def run(inputs, trace=False, **kwargs):
    """Build (cached), run on 8 cores, return (full_output, BassKernelResults)."""
    nc = _get_nc()
    in_maps = _make_in_maps(inputs)
    res = run_bass_kernel_spmd(
        nc, in_maps, core_ids=list(range(N_CORES)), trace=trace, **kwargs
    )
    out = np.concatenate([r["out"] for r in res.results], axis=0)  # [B, T, D]
    return out, res


def kernel(**inputs) -> np.ndarray:
    out, _ = run(inputs, trace=False)
    return out



# revision 5
# speedup vs baseline: 1.1325x; 1.1325x over previous
"""Trainium2 Bass kernel: 2-layer LSTM decoder with embedding lookup.

Reference computation (per nn.Decoder):
    tgt_embed = emb[prev_tgt_tokens]                      # [B, T, D]
    for t in 0..T-1:
        x = tgt_embed[:, t]
        for l in 0..1:
            gates = x @ W_ih[l].T + b_ih[l] + h[l] @ W_hh[l].T + b_hh[l]
            i, f, g, o = split(gates, 4)
            c[l] = sigmoid(f) * c[l] + sigmoid(i) * tanh(g)
            h[l] = sigmoid(o) * tanh(c[l])
            x = h[l]
        out[:, t] = h[1]

Sharding: data-parallel over batch B=64 across 8 cores (8 rows each);
weights replicated; the sequential time loop runs on-device per core.

Kernel design (per core), v2:
  - fp16 operand/activation datatypes throughout (PSUM accumulates fp32);
    rel-err budget is 2e-2, fp16 keeps it ~1e-3.
  - Phase A: embedding gather (indirect DMA) + batched input projection
    x @ W_ih0.T over all T steps at full M=128 PE utilization, spilled to
    a DRAM scratch as fp16.
  - Layer-1 input projections h0 @ W_ih1.T are batched over 16-step
    windows (also M=128), not streamed per step.  Layer 1 lags layer 0 by
    LAG steps; one stacked activation chain serves both layers.
  - Recurrent matmuls use 128x32 column tiling: L0 accumulates into PSUM
    rows 0:8 (array col group 0) while L1 accumulates into rows 32:40
    (group 32) concurrently.  Gate banks f|i and g|o sit side by side in
    two [128,1024] PSUM tiles.
  - The x-projection gx enters PSUM via a tiny identity matmul (K=16)
    instead of DVE adds; the g-gate weights/biases are pre-doubled
    host-side so a single plain sigmoid per tile computes sigma(2 z_g)
    without a per-row scale or an ACT table switch.
  - h -> h^T transposes run on the DMA XBAR (16-bit transpose), off the
    PE/DVE critical engines.
"""

import os

import numpy as np

import concourse.bass as bass
import concourse.mybir as mybir
import concourse.tile as tile
from concourse import bacc
from concourse.bass_utils import run_bass_kernel_spmd
from concourse.masks import make_identity

N_CORES = 8
B = 64
T = int(os.environ.get("BASS_LSTM_T", "128"))
D = 512
V = 32000
G = 2048            # 4*D gate dims per layer
BL = B // N_CORES   # 8 batch rows per core
KC = D // 128       # 4 contraction chunks of 128
MT = BL * T // 128  # M-tiles (128 token rows) for the input matmul
WIN = 16            # wih1 batching window (steps)
LAG = 18            # layer-1 step lag behind layer 0
R1 = 32             # partition row where the layer-1 lane starts
NR = R1 + BL        # 40
SR = 48             # hst rows padded for the DMA XBAR transpose (16-mult)
F32 = mybir.dt.float32
F16 = mybir.dt.float16
I32 = mybir.dt.int32
AFT = mybir.ActivationFunctionType

FSL = slice(0, 512)        # f (in pFI) / g (in pGO) columns
ISL = slice(512, 1024)     # i (in pFI) / o (in pGO) columns


def _build():
    nc = bacc.Bacc(
        "TRN2",
        target_bir_lowering=False,
        debug=False,
        enable_asserts=False,
        num_devices=N_CORES,
    )

    tok_d = nc.dram_tensor("tokens", [BL * T, 1], I32, kind="ExternalInput")
    emb_d = nc.dram_tensor("emb", [V, D], F32, kind="ExternalInput")
    wih0_d = nc.dram_tensor("wih0t", [D, G], F16, kind="ExternalInput")
    whh0_d = nc.dram_tensor("whh0t", [D, G], F16, kind="ExternalInput")
    wih1_d = nc.dram_tensor("wih1t", [D, G], F16, kind="ExternalInput")
    whh1_d = nc.dram_tensor("whh1t", [D, G], F16, kind="ExternalInput")
    bias0_d = nc.dram_tensor("bias0", [128, G], F16, kind="ExternalInput")
    bias1_d = nc.dram_tensor("bias1", [1, G], F16, kind="ExternalInput")
    il_d = nc.dram_tensor("il", [16, 64], F16, kind="ExternalInput")
    ht_d = nc.dram_tensor("ht_init", [128, KC * 48], F16, kind="ExternalInput")
    c_d = nc.dram_tensor("c_init", [NR, D], F16, kind="ExternalInput")
    out_d = nc.dram_tensor("out", [BL, T, D], F16, kind="ExternalOutput")

    with tile.TileContext(nc) as tc:
        _body(
            tc,
            tok=tok_d.ap(),
            emb=emb_d.ap(),
            w=[wih0_d.ap(), whh0_d.ap(), wih1_d.ap(), whh1_d.ap()],
            bias0=bias0_d.ap(),
            bias1=bias1_d.ap(),
            il=il_d.ap(),
            ht0=ht_d.ap(),
            c0=c_d.ap(),
            out=out_d.ap(),
        )
    nc.compile()
    return nc


def _body(tc, tok, emb, w, bias0, bias1, il, ht0, c0, out):
    nc = tc.nc
    with (
        tc.tile_pool(name="wpool", bufs=1) as wp,
        tc.tile_pool(name="dram", bufs=1, space="DRAM") as dr,
        tc.tile_pool(name="state", bufs=1) as st,
        tc.tile_pool(name="work", bufs=2) as wk,
        tc.tile_pool(name="pspool", bufs=1, space="PSUM") as pp,
    ):
        # ---- persistent tiles -------------------------------------------
        id_sb = wp.tile([128, 128], F32)
        make_identity(nc, id_sb[:])

        whh0_sb = wp.tile([128, KC * G], F16)
        wih1_sb = wp.tile([128, KC * G], F16)
        whh1_sb = wp.tile([128, KC * G], F16)
        for dst, src in ((whh0_sb, w[1]), (wih1_sb, w[2]), (whh1_sb, w[3])):
            for c in range(KC):
                nc.sync.dma_start(
                    out=dst[:, c * G : (c + 1) * G],
                    in_=src.rearrange("(c p) n -> p c n", p=128)[:, c, :],
                )

        bias1_sb = wp.tile([1, G], F16)
        nc.sync.dma_start(out=bias1_sb[:], in_=bias1)
        il_sb = wp.tile([16, 64], F16)
        nc.sync.dma_start(out=il_sb[:], in_=il)
        ones_sb = wp.tile([1, 128], F16)
        nc.vector.memset(ones_sb[:], 1.0)

        gx_dram = dr.tile([BL * T, G], F16)

        # LSTM state + chain tiles: L0 lane on rows 0:8, L1 on rows 32:40.
        cst = st.tile([NR, D], F16)
        fct = st.tile([NR, D], F16)
        mt_ = st.tile([NR, D], F16)
        tch = st.tile([NR, D], F16)
        gtFI = st.tile([NR, 2 * D], F16)
        gtGO = st.tile([NR, 2 * D], F16)
        hst = st.tile([SR, D], F16)
        for tile_ in (cst, fct, mt_, tch, gtFI, gtGO, hst):
            nc.vector.memset(tile_[:], 0.0)
        nc.sync.dma_start(out=cst[:BL, :], in_=c0[:BL, :])

        xb = [st.tile([128, KC * 48], F16, name=f"xb{i}") for i in range(2)]
        nc.vector.memset(xb[1][:], 0.0)
        nc.sync.dma_start(out=xb[0][:], in_=ht0)
        winT = [st.tile([128, KC * 128], F16, name=f"winT{i}") for i in range(2)]
        gx1buf = [st.tile([128, G], F16, name=f"gx1buf{i}") for i in range(2)]

        # ---- phase A: gather + transpose + batched input projection ----
        with tc.tile_pool(name="ph0", bufs=1) as p0:
            wih0_sb = p0.tile([128, KC * G], F16)
            for c in range(KC):
                nc.sync.dma_start(
                    out=wih0_sb[:, c * G : (c + 1) * G],
                    in_=w[0].rearrange("(c p) n -> p c n", p=128)[:, c, :],
                )
            bias0_bc = p0.tile([128, G], F16)
            nc.sync.dma_start(out=bias0_bc[:], in_=bias0)

            for m in range(MT):
                idx_m = p0.tile([128, 1], I32, tag="idx", bufs=2)
                nc.sync.dma_start(out=idx_m[:], in_=tok[m * 128 : (m + 1) * 128, :])
                emb_m = p0.tile([128, D], F32, tag="embrows", bufs=2)
                nc.gpsimd.indirect_dma_start(
                    out=emb_m[:],
                    out_offset=None,
                    in_=emb,
                    in_offset=bass.IndirectOffsetOnAxis(ap=idx_m[:, :1], axis=0),
                )
                pst = pp.tile([128, 512], F32, tag="ps", bufs=2, name="pst")
                for c in range(KC):
                    nc.tensor.transpose(
                        out=pst[:, c * 128 : (c + 1) * 128],
                        in_=emb_m[:, c * 128 : (c + 1) * 128],
                        identity=id_sb[:],
                    )
                embT_m = p0.tile([128, D], F16, tag="embT", bufs=2)
                nc.vector.tensor_copy(out=embT_m[:], in_=pst[:, 0:512])
                gxs = p0.tile([128, G], F16, tag="gxs", bufs=2)
                for n in range(4):
                    psm = pp.tile([128, 512], F32, tag="ps", bufs=2, name="psm")
                    for c in range(KC):
                        nc.tensor.matmul(
                            out=psm[:, 0:512],
                            lhsT=embT_m[:, c * 128 : (c + 1) * 128],
                            rhs=wih0_sb[:, c * G + n * 512 : c * G + (n + 1) * 512],
                            start=(c == 0),
                            stop=(c == KC - 1),
                        )
                    nc.vector.tensor_add(
                        out=gxs[:, n * 512 : (n + 1) * 512],
                        in0=psm[:, 0:512],
                        in1=bias0_bc[:, n * 512 : (n + 1) * 512],
                    )
                nc.sync.dma_start(
                    out=gx_dram[m * 128 : (m + 1) * 128, :], in_=gxs[:]
                )

        # ---- phase B: recurrent loop ------------------------------------
        for k in range(T + LAG):
            l0 = k < T
            l1 = k >= LAG
            t = k
            j = k - LAG

            # gx stacks: rows 0:8 = L0 step t, rows 8:16 = L1 step j.
            gsFI = wk.tile([16, 1024], F16, tag="gsFI", bufs=3, name="gsFI")
            gsGO = wk.tile([16, 1024], F16, tag="gsGO", bufs=3, name="gsGO")
            if l0:
                nc.sync.dma_start(
                    out=gsFI[0:8, :], in_=gx_dram[t * 8 : (t + 1) * 8, 0:1024]
                )
                nc.sync.dma_start(
                    out=gsGO[0:8, :], in_=gx_dram[t * 8 : (t + 1) * 8, 1024:2048]
                )
            if l1:
                wbuf = gx1buf[(j // WIN) % 2]
                r = (j % WIN) * 8
                nc.sync.dma_start(out=gsFI[8:16, :], in_=wbuf[r : r + 8, 0:1024])
                nc.sync.dma_start(out=gsGO[8:16, :], in_=wbuf[r : r + 8, 1024:2048])

            xb_prev = xb[k % 2]
            xb_new = xb[(k + 1) % 2]

            pFI = pp.tile([128, 1024], F32, tag="pFI", name="pFI")
            pGO = pp.tile([128, 1024], F32, tag="pGO", name="pGO")
            # inject gx (+biases) into PSUM rows {0:8, 32:40}; start=True
            # clears each 512-col bank before the whh accumulation.
            for p, gs in ((pFI, gsFI), (pGO, gsGO)):
                for cs in (FSL, ISL):
                    nc.tensor.matmul(
                        out=p[0:NR, cs],
                        lhsT=il_sb[0:16, 0:NR],
                        rhs=gs[0:16, cs],
                        start=True,
                        stop=False,
                        skip_group_check=True,
                    )
            # recurrent matmuls, 2-wide column-tiled (groups 0 and 32)
            lanes = []
            if l0:
                lanes.append((whh0_sb, 0))
            if l1:
                lanes.append((whh1_sb, R1))
            for p, goff in ((pFI, 0), (pGO, 1024)):
                for gi, cs in ((0, FSL), (1, ISL)):
                    col = goff + gi * 512
                    for li, (wsb, r0) in enumerate(lanes):
                        for c in range(KC):
                            nc.tensor.matmul(
                                out=p[r0 : r0 + 8, cs],
                                lhsT=xb_prev[:, c * 48 + r0 : c * 48 + r0 + 8],
                                rhs=wsb[:, c * G + col : c * G + col + 512],
                                start=False,
                                stop=(
                                    c == KC - 1
                                    and gi == 1
                                    and li == len(lanes) - 1
                                ),
                                skip_group_check=True,
                            )

            # stacked activation chain (both lanes, rows 0:40)
            nc.scalar.activation(out=gtFI[:, :], in_=pFI[0:NR, :], func=AFT.Sigmoid)
            nc.scalar.activation(out=gtGO[:, :], in_=pGO[0:NR, :], func=AFT.Sigmoid)
            nc.vector.tensor_mul(out=fct[:], in0=gtFI[:, FSL], in1=cst[:])
            nc.vector.tensor_sub(out=fct[:], in0=fct[:], in1=gtFI[:, ISL])
            nc.vector.scalar_tensor_tensor(
                out=mt_[:], in0=gtGO[:, FSL], scalar=2.0, in1=gtFI[:, ISL],
                op0=mybir.AluOpType.mult, op1=mybir.AluOpType.mult,
            )
            nc.vector.tensor_add(out=cst[:], in0=fct[:], in1=mt_[:])
            nc.scalar.activation(out=tch[:], in_=cst[:], func=AFT.Sigmoid, scale=2.0)
            nc.vector.tensor_scalar(
                out=tch[:], in0=tch[:], scalar1=2.0, scalar2=-1.0,
                op0=mybir.AluOpType.mult, op1=mybir.AluOpType.add,
            )
            nc.vector.tensor_mul(out=hst[0:NR, :], in0=gtGO[:, ISL], in1=tch[:])
            if l1:
                nc.sync.dma_start(out=out[:, j, :], in_=hst[R1:NR, :])

            # h -> h^T via DMA XBAR transpose (fp16), both lanes at once
            for c in range(KC):
                nc.sync.dma_start(
                    out=xb_new[:, c * 48 : (c + 1) * 48],
                    in_=hst[0:SR, c * 128 : (c + 1) * 128],
                    transpose=True,
                )

            if k == LAG - 1:
                # splice in layer-1 initial state before iteration LAG
                for c in range(KC):
                    nc.sync.dma_start(
                        out=xb_new[:, c * 48 + R1 : c * 48 + NR],
                        in_=ht0[:, c * 48 + R1 : c * 48 + NR],
                    )
                nc.sync.dma_start(out=cst[R1:NR, :], in_=c0[R1:NR, :])

            if l0:
                # collect h0^T into the wih1 window operand
                wT = winT[(t // WIN) % 2]
                s = t % WIN
                for c in range(KC):
                    nc.sync.dma_start(
                        out=wT[:, c * 128 + s * 8 : c * 128 + (s + 1) * 8],
                        in_=xb_new[:, c * 48 : c * 48 + 8],
                    )
                if s == WIN - 1:
                    # batched wih1 projection for this window (M=128)
                    wbuf = gx1buf[(t // WIN) % 2]
                    for n in range(4):
                        pw = pp.tile([128, 512], F32, tag="pw", name="pw")
                        nc.tensor.matmul(
                            out=pw[:, 0:512],
                            lhsT=ones_sb[0:1, 0:128],
                            rhs=bias1_sb[0:1, n * 512 : (n + 1) * 512],
                            start=True,
                            stop=False,
                            skip_group_check=True,
                        )
                        for c in range(KC):
                            nc.tensor.matmul(
                                out=pw[:, 0:512],
                                lhsT=wT[:, c * 128 : (c + 1) * 128],
                                rhs=wih1_sb[:, c * G + n * 512 : c * G + (n + 1) * 512],
                                start=False,
                                stop=(c == KC - 1),
                                skip_group_check=True,
                            )
                        nc.scalar.copy(
                            out=wbuf[:, n * 512 : (n + 1) * 512], in_=pw[:, 0:512]
                        )


_NC_CACHE = {}


def _get_nc():
    if "nc" not in _NC_CACHE:
        _NC_CACHE["nc"] = _build()
    return _NC_CACHE["nc"]


def _make_in_maps(inputs):
    tokens = np.asarray(inputs["prev_tgt_tokens"])[:, :T].astype(np.int32)  # [B, T]
    emb = np.ascontiguousarray(np.asarray(inputs["emb"], dtype=np.float32))
    W_ih = np.asarray(inputs["W_ih"], dtype=np.float32)
    W_hh = np.asarray(inputs["W_hh"], dtype=np.float32)
    b_ih = np.asarray(inputs["b_ih"], dtype=np.float32)
    b_hh = np.asarray(inputs["b_hh"], dtype=np.float32)
    hiddens = np.asarray(inputs["hiddens"], dtype=np.float32)
    cells = np.asarray(inputs["cells"], dtype=np.float32)

    def permute_gates(a, axis):
        # PyTorch order [i, f, g, o] -> kernel order [f, i, g, o]; the g
        # block is doubled so a plain sigmoid computes sigma(2 z_g).
        i, f, g, o = np.split(a, 4, axis=axis)
        return np.concatenate([f, i, 2.0 * g, o], axis=axis)

    wih0t = permute_gates(W_ih[0].T, 1).astype(np.float16)  # [D, G]
    whh0t = permute_gates(W_hh[0].T, 1).astype(np.float16)
    wih1t = permute_gates(W_ih[1].T, 1).astype(np.float16)
    whh1t = permute_gates(W_hh[1].T, 1).astype(np.float16)
    bias0 = np.ascontiguousarray(
        np.broadcast_to(
            permute_gates(b_ih[0] + b_hh[0], 0)[None, :], (128, G)
        ).astype(np.float16)
    )
    bias1 = permute_gates(b_ih[1] + b_hh[1], 0)[None, :].astype(np.float16)

    il = np.zeros((16, 64), np.float16)
    for r in range(8):
        il[r, r] = 1.0
        il[8 + r, R1 + r] = 1.0

    in_maps = []
    for core in range(N_CORES):
        sl = slice(core * BL, (core + 1) * BL)
        tok_tm = np.ascontiguousarray(tokens[sl].T.reshape(BL * T, 1))  # t-major
        ht = np.zeros((128, KC * 48), np.float16)
        for c in range(KC):
            ht[:, c * 48 : c * 48 + 8] = hiddens[0, sl, c * 128 : (c + 1) * 128].T
            ht[:, c * 48 + R1 : c * 48 + NR] = hiddens[1, sl, c * 128 : (c + 1) * 128].T
        cin = np.zeros((NR, D), np.float16)
        cin[0:BL] = cells[0, sl]
        cin[R1:NR] = cells[1, sl]
        in_maps.append(
            {
                "tokens": tok_tm,
                "emb": emb,
                "wih0t": np.ascontiguousarray(wih0t),
                "whh0t": np.ascontiguousarray(whh0t),
                "wih1t": np.ascontiguousarray(wih1t),
                "whh1t": np.ascontiguousarray(whh1t),
                "bias0": bias0,
                "bias1": np.ascontiguousarray(bias1),
                "il": il,
                "ht_init": ht,
                "c_init": cin,
            }
        )
    return in_maps


def run(inputs, trace=False, **kwargs):
    """Build (cached), run on 8 cores, return (full_output, BassKernelResults)."""
    nc = _get_nc()
    in_maps = _make_in_maps(inputs)
    res = run_bass_kernel_spmd(
        nc, in_maps, core_ids=list(range(N_CORES)), trace=trace, **kwargs
    )
    out = np.concatenate([r["out"] for r in res.results], axis=0)  # [B, T, D]
    return out.astype(np.float32), res


def kernel(**inputs) -> np.ndarray:
    out, _ = run(inputs, trace=False)
    return out
